# revision 4
# baseline (speedup 1.0000x reference)
"""Trainium2 Bass kernel for Llama4TextExperts (MoE expert MLP chain).

Problem: E=8 experts, T=2048 tokens/expert, H=2048 hidden, D=4096 intermediate.
  hs (E*T, H) -> per expert e: g = hs_e @ Wg_e; u = hs_e @ Wu_e;
  f = u * silu(g); y_e = f @ Wd_e  -> out (E*T, H), all fp32.

Sharding: expert-parallel, 1 expert per NeuronCore (8 cores).

Per-core kernel design (v2):
  - All matmul operands bf16 (error ~3e-3 vs fp64, threshold 2e-2).
  - Host pre-transposes hs_e -> xT [H, T]; all of xT resident in SBUF
    (8.4MB), loaded once in 8 chunked DMAs.
  - Loop over T in tiles of TT=1024 tokens:
      stage 1: per d-tile (128 wide): psum_g/psum_u [128, 1024] (2 PSUM
        banks) accumulate 16 compound matmuls over h-chunks (lhsT =
        W[h,d] 128x128 stationary reused across the 1024-token moving
        stream -> 1 LDWEIGHTS per 2 hw matmuls). silu on ScalarE,
        f = silu(g)*u on VectorE -> f[dt] SBUF [128(d) x 1024(t)] bf16.
      stage 2: computed as y^T: for each 128-wide h-block, psum_y
        [128(h) x 1024(t)] accumulates 32 compound matmuls over d
        (lhsT = wd[d,h] 128x128 stationary, rhs = f[dt] 128x1024
        moving). Evacuate via ScalarE -> DMA to y [H, T] (y^T layout;
        host transposes back).
  - Total per-core DMA ~92MB (vs 285MB for the f32-wd variant); weight
    loads stream during compute via double-buffered pools.
"""

import os
import sys

for _p in ("/opt/trn_rl_repo",):
    if _p not in sys.path and os.path.isdir(_p):
        sys.path.insert(0, _p)

import numpy as np
from ml_dtypes import bfloat16 as bf16

E = 8
T = 2048
H = 2048
D = 4096

_CACHE = {}


def _build_bass(H_=H, D_=D, T_=T, TT=1024):
    """Build the single-core Bass module (same program for all 8 cores)."""
    import concourse.bass as bass
    import concourse.mybir as mybir
    from concourse.tile import TileContext

    f32 = mybir.dt.float32
    bf = mybir.dt.bfloat16
    P = 128
    N_H = H_ // P            # h-chunks (16)
    N_D = D_ // P            # d-tiles (32)
    N_TT = T_ // TT          # t-tiles (2)
    WGD = 256                # wg/wu d-width per load (2 d-tiles)

    nc = bass.Bass(trn_type="TRN2")

    xT = nc.declare_dram_parameter("xT", [H_, T_], bf, isOutput=False)
    wg = nc.declare_dram_parameter("wg", [H_, D_], bf, isOutput=False)
    wu = nc.declare_dram_parameter("wu", [H_, D_], bf, isOutput=False)
    wd = nc.declare_dram_parameter("wd", [D_, H_], bf, isOutput=False)
    y = nc.declare_dram_parameter("y", [H_, T_], f32, isOutput=True)  # y^T

    xT_r = xT[:].rearrange("(n p) t -> p n t", p=P)    # [128, N_H, T]
    wg_r = wg[:].rearrange("(n p) d -> p n d", p=P)    # [128, N_H, D]
    wu_r = wu[:].rearrange("(n p) d -> p n d", p=P)
    wd_r = wd[:].rearrange("(n p) h -> p n h", p=P)    # [128, N_D, H]
    y_r = y[:].rearrange("(n p) t -> p n t", p=P)      # [128, N_H, T]

    with TileContext(nc) as tc:
        with (
            tc.tile_pool(name="xpool", bufs=1) as xpool,
            tc.tile_pool(name="wpool", bufs=2) as wpool,
            tc.tile_pool(name="wdpool", bufs=2) as wdpool,
            tc.tile_pool(name="fpool", bufs=N_D) as fpool,
            tc.tile_pool(name="spool", bufs=2) as spool,
            tc.tile_pool(name="ypool", bufs=4) as ypool,
            tc.tile_pool(name="pgu", bufs=1, space="PSUM") as pgu,
            tc.tile_pool(name="py", bufs=2, space="PSUM") as py,
        ):
            # ---- load ALL of xT once, in 8 chunks for DMA pipelining
            x_all = xpool.tile([P, N_H, T_], bf, tag="x")
            for c in range(8):
                nc.sync.dma_start(
                    out=x_all[:, 2 * c:2 * c + 2, :],
                    in_=xT_r[:, 2 * c:2 * c + 2, :],
                )

            for tt in range(N_TT):
                tsl = slice(tt * TT, (tt + 1) * TT)

                # ---- stage 1: gate/up + swiglu, d-tile at a time
                f_tiles = []
                for dt in range(N_D):
                    dw = dt % (WGD // P)   # position inside current weight load
                    if dw == 0:
                        dsl = slice(dt * P, dt * P + WGD)
                        wg_t = wpool.tile([P, N_H, WGD], bf, tag="wg")
                        wu_t = wpool.tile([P, N_H, WGD], bf, tag="wu")
                        nc.sync.dma_start(out=wg_t, in_=wg_r[:, :, dsl])
                        nc.sync.dma_start(out=wu_t, in_=wu_r[:, :, dsl])
                    psum_g = pgu.tile([P, TT], f32, tag="pg")
                    psum_u = pgu.tile([P, TT], f32, tag="pu")
                    HB = TT // 2   # one-PSUM-bank wide matmul halves
                    for h in range(N_H):
                        for b in range(2):
                            nc.tensor.matmul(
                                psum_g[:, b * HB:(b + 1) * HB],
                                lhsT=wg_t[:, h, dw * P:(dw + 1) * P],
                                rhs=x_all[:, h, tt * TT + b * HB:
                                          tt * TT + (b + 1) * HB],
                                start=(h == 0), stop=(h == N_H - 1),
                            )
                    for h in range(N_H):
                        for b in range(2):
                            nc.tensor.matmul(
                                psum_u[:, b * HB:(b + 1) * HB],
                                lhsT=wu_t[:, h, dw * P:(dw + 1) * P],
                                rhs=x_all[:, h, tt * TT + b * HB:
                                          tt * TT + (b + 1) * HB],
                                start=(h == 0), stop=(h == N_H - 1),
                            )
                    s_t = spool.tile([P, TT], f32, tag="s")
                    nc.scalar.activation(
                        out=s_t, in_=psum_g,
                        func=mybir.ActivationFunctionType.Silu,
                    )
                    f_t = fpool.tile([P, TT], bf, tag="f")
                    nc.vector.tensor_mul(f_t, s_t, psum_u)
                    f_tiles.append(f_t)

                # ---- stage 2: y^T[hb] = sum_dt wd[dt, hb].T @ f[dt]
                for hb in range(N_H):
                    wd_t = wdpool.tile([P, N_D, P], bf, tag="wd")
                    nc.sync.dma_start(
                        out=wd_t,
                        in_=wd_r[:, :, hb * P:(hb + 1) * P],
                    )
                    psum_y = py.tile([P, TT], f32, tag="py")
                    HB = TT // 2
                    for dt in range(N_D):
                        for b in range(2):
                            nc.tensor.matmul(
                                psum_y[:, b * HB:(b + 1) * HB],
                                lhsT=wd_t[:, dt, :],
                                rhs=f_tiles[dt][:, b * HB:(b + 1) * HB],
                                start=(dt == 0), stop=(dt == N_D - 1),
                            )
                    for half in range(2):
                        y_sb = ypool.tile([P, TT // 2], f32, tag="y")
                        nc.scalar.copy(
                            out=y_sb,
                            in_=psum_y[:, half * (TT // 2):(half + 1) * (TT // 2)],
                        )
                        nc.sync.dma_start(
                            out=y_r[:, hb,
                                    tt * TT + half * (TT // 2):
                                    tt * TT + (half + 1) * (TT // 2)],
                            in_=y_sb,
                        )
    _split_matmul_waits(nc)
    return nc


def _split_matmul_waits(nc):
    """walrus splits Matmult into LDW+MM and moves the Matmult's sync
    waits onto the generated LW struct, which has room for only one wait.
    Hoist every Matmult's waits onto a PE InstNoOp inserted just before it."""
    import concourse.mybir as mybir

    for f in nc.m.functions:
        for bb in f.blocks:
            insts = list(bb.instructions)
            out = []
            n_nops = 0
            for ins in insts:
                si = ins.sync_info
                tname = type(ins).__name__
                if (
                    si is not None
                    and len(si.on_wait) > (1 if tname != "InstMatmult" else 0)
                ):
                    keep = [] if tname == "InstMatmult" else [si.on_wait[-1]]
                    hoist = si.on_wait if tname == "InstMatmult" else si.on_wait[:-1]
                    for i, w in enumerate(hoist):
                        nop = mybir.InstNoOp(
                            name=f"{ins.name}-waitnop{i}",
                            engine=ins.engine,
                            ins=[],
                            outs=[],
                            sync_info=mybir.SyncInfo(
                                on_wait=[w], on_update=[]
                            ),
                        )
                        out.append(nop)
                        n_nops += 1
                    ins.sync_info = mybir.SyncInfo(
                        on_wait=keep, on_update=list(si.on_update)
                    )
                out.append(ins)
            if n_nops:
                bb.instructions = out


def make_in_maps(hidden_states, gate_proj, up_proj, down_proj):
    hs = np.ascontiguousarray(hidden_states, dtype=np.float32).reshape(E, T, H)
    in_maps = []
    for e in range(E):
        in_maps.append({
            "xT": np.ascontiguousarray(hs[e].T).astype(bf16),
            "wg": np.ascontiguousarray(gate_proj[e], dtype=np.float32).astype(bf16),
            "wu": np.ascontiguousarray(up_proj[e], dtype=np.float32).astype(bf16),
            "wd": np.ascontiguousarray(down_proj[e], dtype=np.float32).astype(bf16),
        })
    return in_maps


def kernel(hidden_states, gate_proj, up_proj, down_proj):
    from concourse.bass_utils import run_bass_kernel_spmd

    in_maps = make_in_maps(hidden_states, gate_proj, up_proj, down_proj)
    if "nc" not in _CACHE:
        _CACHE["nc"] = _build_bass()
    nc = _CACHE["nc"]

    res = run_bass_kernel_spmd(nc, in_maps, core_ids=list(range(E)))
    # y comes back as y^T [H, T] per expert
    out = np.concatenate(
        [np.ascontiguousarray(res.results[e]["y"].T) for e in range(E)], axis=0
    )
    return out.astype(np.float32)


if __name__ == "__main__":
    # smoke: build only
    nc = _build_bass()
    print("built ok, instructions:", len(nc.inst_map))


# revision 5
# speedup vs baseline: 1.1783x; 1.1783x over previous
"""Trainium2 Bass kernel for Llama4TextExperts (MoE expert MLP chain).

Problem: E=8 experts, T=2048 tokens/expert, H=2048 hidden, D=4096 intermediate.
  hs (E*T, H) -> per expert e: g = hs_e @ Wg_e; u = hs_e @ Wu_e;
  f = u * silu(g); y_e = f @ Wd_e  -> out (E*T, H), all fp32.

Sharding: expert-parallel, 1 expert per NeuronCore (8 cores).

Per-core kernel design (v3):
  - All matmul operands bf16 (measured rel err ~3.7e-3 vs fp64; gate 2e-2).
  - Host pre-transposes hs_e -> xT [H, T]; all of xT resident in SBUF
    (8.4MB), chunk-DMAed so the first t-tile's slices land first.
  - Loop over T in tiles of TT=512 tokens (one PSUM bank per matmul,
    contiguous accumulation groups -- bank alternation between
    consecutive matmuls measurably breaks LDWEIGHTS pull-ahead):
      stage 1: per d-tile (128 wide): psum_g/psum_u [128, 512] accumulate
        16 matmuls over h-chunks (lhsT = W[h,d] 128x128 stationary,
        rhs = xT[h, t-tile] 128x512 moving). silu on ScalarE,
        f = silu(g)*u on VectorE -> f[dt] SBUF [128(d) x 512(t)] bf16.
      stage 2: computed as y^T: per 128-wide h-block, psum_y [128(h) x
        512(t)] accumulates 32 matmuls over d (lhsT = wd[d,h] 128x128
        stationary, rhs = f[dt] 128x512 moving). ScalarE copy -> DMA to
        y [H, T] (y^T layout; host transposes back).
  - Per-core DMA ~160MB total; weight streams double-buffered under
    compute.
"""

import os
import sys

for _p in ("/opt/trn_rl_repo",):
    if _p not in sys.path and os.path.isdir(_p):
        sys.path.insert(0, _p)

import numpy as np
from ml_dtypes import bfloat16 as bf16

E = 8
T = 2048
H = 2048
D = 4096

_CACHE = {}


def _build_bass(H_=H, D_=D, T_=T, TT=512):
    """Build the single-core Bass module (same program for all 8 cores)."""
    import concourse.bass as bass
    import concourse.mybir as mybir
    from concourse.tile import TileContext

    f32 = mybir.dt.float32
    bf = mybir.dt.bfloat16
    P = 128
    N_H = H_ // P            # h-chunks (16)
    N_D = D_ // P            # d-tiles (32)
    N_TT = T_ // TT          # t-tiles (4)
    WGD = 256                # wg/wu d-width per load (2 d-tiles)

    nc = bass.Bass(trn_type="TRN2")

    xT = nc.declare_dram_parameter("xT", [H_, T_], bf, isOutput=False)
    wg = nc.declare_dram_parameter("wg", [H_, D_], bf, isOutput=False)
    wu = nc.declare_dram_parameter("wu", [H_, D_], bf, isOutput=False)
    wd = nc.declare_dram_parameter("wd", [D_, H_], bf, isOutput=False)
    y = nc.declare_dram_parameter("y", [H_, T_], f32, isOutput=True)  # y^T

    xT_r = xT[:].rearrange("(n p) t -> p n t", p=P)    # [128, N_H, T]
    wg_r = wg[:].rearrange("(n p) d -> p n d", p=P)    # [128, N_H, D]
    wu_r = wu[:].rearrange("(n p) d -> p n d", p=P)
    wd_r = wd[:].rearrange("(n p) h -> p n h", p=P)    # [128, N_D, H]
    y_r = y[:].rearrange("(n p) t -> p n t", p=P)      # [128, N_H, T]

    with TileContext(nc) as tc:
        with (
            tc.tile_pool(name="xpool", bufs=1) as xpool,
            tc.tile_pool(name="wpool", bufs=2) as wpool,
            tc.tile_pool(name="wdpool", bufs=2) as wdpool,
            tc.tile_pool(name="fpool", bufs=N_D) as fpool,
            tc.tile_pool(name="spool", bufs=2) as spool,
            tc.tile_pool(name="ypool", bufs=4) as ypool,
            tc.tile_pool(name="pgu", bufs=2, space="PSUM") as pgu,
            tc.tile_pool(name="py", bufs=4, space="PSUM") as py,
        ):
            # ---- load ALL of xT once; first t-tile's slices first so
            # stage 1 can start ~6us in.
            x_all = xpool.tile([P, N_H, T_], bf, tag="x")
            for tc_ in range(N_TT):
                for hh in range(2):
                    nc.sync.dma_start(
                        out=x_all[:, hh * 8:(hh + 1) * 8,
                                  tc_ * TT:(tc_ + 1) * TT],
                        in_=xT_r[:, hh * 8:(hh + 1) * 8,
                                 tc_ * TT:(tc_ + 1) * TT],
                    )

            for tt in range(N_TT):
                tsl = slice(tt * TT, (tt + 1) * TT)

                # ---- stage 1: gate/up + swiglu, d-tile at a time
                f_tiles = []
                for dt in range(N_D):
                    dw = dt % (WGD // P)   # position inside current weight load
                    if dw == 0:
                        dsl = slice(dt * P, dt * P + WGD)
                        wg_t = wpool.tile([P, N_H, WGD], bf, tag="wg")
                        wu_t = wpool.tile([P, N_H, WGD], bf, tag="wu")
                        nc.sync.dma_start(out=wg_t, in_=wg_r[:, :, dsl])
                        nc.sync.dma_start(out=wu_t, in_=wu_r[:, :, dsl])
                    psum_g = pgu.tile([P, TT], f32, tag="pg")
                    psum_u = pgu.tile([P, TT], f32, tag="pu")
                    for h in range(N_H):
                        nc.tensor.matmul(
                            psum_g,
                            lhsT=wg_t[:, h, dw * P:(dw + 1) * P],
                            rhs=x_all[:, h, tsl],
                            start=(h == 0), stop=(h == N_H - 1),
                        )
                    for h in range(N_H):
                        nc.tensor.matmul(
                            psum_u,
                            lhsT=wu_t[:, h, dw * P:(dw + 1) * P],
                            rhs=x_all[:, h, tsl],
                            start=(h == 0), stop=(h == N_H - 1),
                        )
                    s_t = spool.tile([P, TT], f32, tag="s")
                    nc.scalar.activation(
                        out=s_t, in_=psum_g,
                        func=mybir.ActivationFunctionType.Silu,
                    )
                    f_t = fpool.tile([P, TT], bf, tag="f")
                    nc.vector.tensor_mul(f_t, s_t, psum_u)
                    f_tiles.append(f_t)

                # ---- stage 2: y^T[hb] = sum_dt wd[dt, hb].T @ f[dt]
                for hb in range(N_H):
                    wd_t = wdpool.tile([P, N_D, P], bf, tag="wd")
                    nc.sync.dma_start(
                        out=wd_t,
                        in_=wd_r[:, :, hb * P:(hb + 1) * P],
                    )
                    psum_y = py.tile([P, TT], f32, tag="py")
                    for dt in range(N_D):
                        nc.tensor.matmul(
                            psum_y,
                            lhsT=wd_t[:, dt, :],
                            rhs=f_tiles[dt][:, :],
                            start=(dt == 0), stop=(dt == N_D - 1),
                        )
                    y_sb = ypool.tile([P, TT], f32, tag="y")
                    nc.scalar.copy(out=y_sb, in_=psum_y)
                    nc.sync.dma_start(out=y_r[:, hb, tsl], in_=y_sb)
    _split_matmul_waits(nc)
    return nc


def _split_matmul_waits(nc):
    """walrus splits Matmult into LDW+MM and moves the Matmult's sync
    waits onto the generated LW struct, which has room for only one wait.
    Hoist every Matmult's waits onto a PE InstNoOp inserted just before it."""
    import concourse.mybir as mybir

    for f in nc.m.functions:
        for bb in f.blocks:
            insts = list(bb.instructions)
            out = []
            n_nops = 0
            for ins in insts:
                si = ins.sync_info
                tname = type(ins).__name__
                if (
                    si is not None
                    and len(si.on_wait) > (1 if tname != "InstMatmult" else 0)
                ):
                    keep = [] if tname == "InstMatmult" else [si.on_wait[-1]]
                    hoist = si.on_wait if tname == "InstMatmult" else si.on_wait[:-1]
                    for i, w in enumerate(hoist):
                        nop = mybir.InstNoOp(
                            name=f"{ins.name}-waitnop{i}",
                            engine=ins.engine,
                            ins=[],
                            outs=[],
                            sync_info=mybir.SyncInfo(
                                on_wait=[w], on_update=[]
                            ),
                        )
                        out.append(nop)
                        n_nops += 1
                    ins.sync_info = mybir.SyncInfo(
                        on_wait=keep, on_update=list(si.on_update)
                    )
                out.append(ins)
            if n_nops:
                bb.instructions = out


def make_in_maps(hidden_states, gate_proj, up_proj, down_proj):
    hs = np.ascontiguousarray(hidden_states, dtype=np.float32).reshape(E, T, H)
    in_maps = []
    for e in range(E):
        in_maps.append({
            "xT": np.ascontiguousarray(hs[e].T).astype(bf16),
            "wg": np.ascontiguousarray(gate_proj[e], dtype=np.float32).astype(bf16),
            "wu": np.ascontiguousarray(up_proj[e], dtype=np.float32).astype(bf16),
            "wd": np.ascontiguousarray(down_proj[e], dtype=np.float32).astype(bf16),
        })
    return in_maps


def kernel(hidden_states, gate_proj, up_proj, down_proj):
    from concourse.bass_utils import run_bass_kernel_spmd

    in_maps = make_in_maps(hidden_states, gate_proj, up_proj, down_proj)
    if "nc" not in _CACHE:
        _CACHE["nc"] = _build_bass()
    nc = _CACHE["nc"]

    res = run_bass_kernel_spmd(nc, in_maps, core_ids=list(range(E)))
    # y comes back as y^T [H, T] per expert
    out = np.concatenate(
        [np.ascontiguousarray(res.results[e]["y"].T) for e in range(E)], axis=0
    )
    return out.astype(np.float32)


if __name__ == "__main__":
    # smoke: build only
    nc = _build_bass()
    print("built ok, instructions:", len(nc.inst_map))
